# revision 15
# baseline (speedup 1.0000x reference)
"""Distributed Trainium2 kernel for a GATv2 layer + BN + global-mean-pool + classifier.

Math (reference, heads=1):
    xl = x@Wl + bl ; xr = x@Wr + br
    logit_e = att . leaky_relu(xl[src_e] + xr[dst_e], 0.2)
    a_e     = segment_softmax(logit_e over dst)
    out_i   = sum_{e: dst=i} a_e * xl[src_e] ; out = out + bias1
    h       = BN(out) ; g = mean_i h ; y = softmax(g@Wc + bc)

Only the global mean over nodes matters, so per-node outputs never
materialize:
    y = softmax( ((S/N)*A + B) @ Wc + bc ),  S = sum_e a_e * xl[src_e],
    A = gamma/sqrt(var+eps), B = (bias1 - mu)*A + beta.

Attention weights v = att are folded into the tables host-side:
    v_f * lrelu(z_f) = sign_f * lrelu(|v_f| z_f)
with features permuted so positive-sign features occupy columns [0,PP).

Layout: per core, nodes sorted by in-degree (desc) and processed 128 per
batch, one node per partition; a node's in-edges occupy D consecutive
slot-columns of its partition row (D = max degree in batch; batches with
equal D are grouped into chunks).  Per edge ONE dma_gather fetches the
packed pair row of xl[src] (int16 indices address node pairs); xr[dst] is
a stride-0 broadcast of the node's own row; the self-loop edge is computed
straight from the local tables (no gather).  Segment softmax is a plain
row-reduce per batch.  The weighted sum uses sum_e w_e z_e - sum_d xr_d
(softmax weights sum to 1 per node), accumulated per-column into a
[128, F] accumulator, finished with one ones-matmul + AllReduce + head.
"""

import os

import ml_dtypes
import numpy as np

import concourse.bass as bass
import concourse.bacc as bacc
import concourse.mybir as mybir
import concourse.tile as tile

M = 8  # cores
F = 128
NCLS = 5
BN_EPS = 1e-5
NPCR = 6250     # real nodes per core
NB = 49         # batches of 128 nodes (6272 padded)
NPC = NB * 128
NG = M * NPC
CAP = 32        # max slot-columns per chunk (SBUF budget)
NBC = 16        # max batches per chunk

BF16 = ml_dtypes.bfloat16


def _wrap_idx(seq):
    """[n] int array -> [128, n//16] int16 wrap layout (16-partition groups,
    replicated across the 8 gpsimd cores)."""
    n = seq.shape[0]
    assert n % 16 == 0
    w = seq.reshape(n // 16, 16).T.astype(np.int16)
    return np.tile(w, (8, 1))


def prep_host(x, edge_index, Wl, bl, Wr, br, att, bias1,
              bn_gamma, bn_beta, bn_mean, bn_var, Wc, bc):
    N = x.shape[0]
    assert N == NPCR * M
    src = np.asarray(edge_index[0], np.int64)
    dst = np.asarray(edge_index[1], np.int64)

    # ---- attention folding ----
    v = np.asarray(att[0], np.float64)
    posm = v >= 0
    perm = np.argsort(~posm, kind="stable")
    PP = int(posm.sum())
    assert 0 < PP < F, f"degenerate attention sign split PP={PP}"
    absv = np.abs(v[perm])
    Wg_l = (Wl[:, perm] * absv[None, :]).astype(np.float32)
    bg_l = (bl[perm] * absv).astype(np.float32)
    Wg_r = (Wr[:, perm] * absv[None, :]).astype(np.float32)
    bg_r = (br[perm] * absv).astype(np.float32)

    # ---- per-core degree-sorted node order ----
    deg = np.bincount(dst, minlength=N)  # in-degree excluding self loop
    rank = np.zeros(N, np.int64)         # node -> global padded rank
    xT = np.zeros((M, 128, NPC), BF16)
    smask = np.zeros((M, 128, NB), np.float32)  # real-node mask [p, b]
    Dbs = np.zeros((M, NB), np.int64)
    for k in range(M):
        lo = k * NPCR
        dk = deg[lo:lo + NPCR]
        order = np.argsort(-dk, kind="stable")     # rank -> local node
        rank[lo + order] = k * NPC + np.arange(NPCR)
        xk = np.zeros((NPC, F), np.float32)
        xk[:NPCR] = x[lo + order]
        xT[k] = np.ascontiguousarray(xk.T.astype(BF16))
        r = np.arange(NPC)
        smask[k] = ((r % 128) * 0 + (r < NPCR)).astype(np.float32) \
            .reshape(NB, 128).T
        Dbs[k] = np.concatenate([np.sort(dk)[::-1], np.zeros(NPC - NPCR,
                                np.int64)]).reshape(NB, 128).max(axis=1)

    # per-core chunk schedules must be IDENTICAL (SPMD one program).
    # Use the max D over cores for each batch index.
    Dmax_b = Dbs.max(axis=0)           # [NB] non-increasing? per-core sorted
    Dmax_b = np.maximum.accumulate(Dmax_b[::-1])[::-1]  # enforce non-increasing
    chunks = []   # (b0, nb_c, D, coloff)
    coloff = 0
    b = 0
    while b < NB:
        D = int(Dmax_b[b])
        e = b
        while e < NB and int(Dmax_b[e]) == D:
            e += 1
        run = e - b
        step = max(1, min(NBC, (CAP // D) if D > 0 else NBC))
        while b < e:
            nb_c = min(step, e - b)
            chunks.append((b, nb_c, D, coloff))
            coloff += nb_c * D
            b += nb_c
    TC = coloff  # total gathered columns
    TCpad = ((TC + 1 + 7) // 8) * 8  # pad idx width to mult of 8 cols

    # ---- per-core slot tables ----
    # CSR of in-edges by dst, in rank order
    iP = np.zeros((M, 128, TCpad), np.int64)
    par = np.zeros((M, 128, TCpad), np.float32)
    pmask = np.zeros((M, 128, TCpad), np.float32)
    srcrow = rank[src]
    for k in range(M):
        lo = k * NPCR
        sel = (dst >= lo) & (dst < lo + NPCR)
        d_r = rank[dst[sel]] - k * NPC        # local rank of dst
        s_r = srcrow[sel]                     # global padded rank of src
        o = np.argsort(d_r, kind="stable")
        d_r = d_r[o]
        s_r = s_r[o]
        cnt = np.bincount(d_r, minlength=NPC)
        starts = np.concatenate([[0], np.cumsum(cnt)])
        # slot (p, col) for chunk (b0, nb, D): col = coloff + bi*D + d
        # edge d of node rank (b0+bi)*128 + p
        pos_in_seg = np.arange(len(d_r)) - starts[d_r]
        bnode = d_r // 128
        pnode = d_r % 128
        # find chunk of bnode
        colbase = np.zeros(NB, np.int64)
        Dof = np.zeros(NB, np.int64)
        for (b0, nb_c, D, co) in chunks:
            for bi in range(nb_c):
                colbase[b0 + bi] = co + bi * D
                Dof[b0 + bi] = D
        assert (pos_in_seg < Dof[bnode]).all()
        cols = colbase[bnode] + pos_in_seg
        iP[k, pnode, cols] = s_r >> 1
        par[k, pnode, cols] = (s_r & 1).astype(np.float32)
        pmask[k, pnode, cols] = 1.0

    iP_w = np.stack([
        _wrap_idx(iP[k, :, :TCpad].T.reshape(-1)) for k in range(M)])

    # ---- head constants ----
    A = bn_gamma.astype(np.float64) / np.sqrt(bn_var.astype(np.float64) + BN_EPS)
    Ap = (A[perm] / (N * absv)).astype(np.float32).reshape(F, 1)
    Bp = ((bias1 - bn_mean).astype(np.float64) * A + bn_beta)[perm] \
        .astype(np.float32).reshape(F, 1)
    Wcp = Wc[perm, :].astype(np.float32)

    meta = dict(PP=PP, TC=TC, TCpad=TCpad, chunks=tuple(chunks))

    in_maps = []
    for k in range(M):
        in_maps.append({
            "xT": np.ascontiguousarray(xT[k]),
            "Wgl": Wg_l.astype(BF16),
            "bgl": bg_l.reshape(1, F).astype(BF16),
            "Wgr": Wg_r.astype(BF16),
            "bgr": bg_r.reshape(1, F).astype(BF16),
            "iP": np.ascontiguousarray(iP_w[k]),
            "par": np.ascontiguousarray(par[k]),
            "pmask": np.ascontiguousarray(pmask[k]),
            "smask": np.ascontiguousarray(smask[k]),
            "smaskb": np.ascontiguousarray(smask[k].astype(BF16)),
            "Ap": Ap,
            "Bp": Bp,
            "Wcp": Wcp,
            "bc": bc.reshape(1, NCLS).astype(np.float32),
        })
    return in_maps, meta


def build(meta):
    PP, TC, TCpad = meta["PP"], meta["TC"], meta["TCpad"]
    chunks = meta["chunks"]
    CAPC = max(CAP, max(nb * D for (_, nb, D, _) in chunks))
    LW = (TCpad * 128) // 16

    dt = mybir.dt
    alu = mybir.AluOpType
    act = mybir.ActivationFunctionType
    rg = [list(range(M))]

    nc = bacc.Bacc("TRN2", target_bir_lowering=False, debug=False, num_devices=M)

    def p_in(name, shape, d):
        return nc.dram_tensor(name, shape, d, kind="ExternalInput").ap()

    xT = p_in("xT", [128, NPC], dt.bfloat16)
    Wgl = p_in("Wgl", [F, F], dt.bfloat16)
    bgl = p_in("bgl", [1, F], dt.bfloat16)
    Wgr = p_in("Wgr", [F, F], dt.bfloat16)
    bgr = p_in("bgr", [1, F], dt.bfloat16)
    iP = p_in("iP", [128, LW], dt.int16)
    par = p_in("par", [128, TCpad], dt.float32)
    pmask = p_in("pmask", [128, TCpad], dt.float32)
    smask = p_in("smask", [128, NB], dt.float32)
    smaskb = p_in("smaskb", [128, NB], dt.bfloat16)
    Ap = p_in("Ap", [F, 1], dt.float32)
    Bp = p_in("Bp", [F, 1], dt.float32)
    Wcp = p_in("Wcp", [F, NCLS], dt.float32)
    bc = p_in("bc", [1, NCLS], dt.float32)
    out = nc.dram_tensor("out", [1, NCLS], dt.float32, kind="ExternalOutput").ap()

    with tile.TileContext(nc) as tc:
        with (
            tc.tile_pool(name="dram", bufs=1, space="DRAM") as dpool,
            tc.tile_pool(name="sbp", bufs=1) as sbp,
            tc.tile_pool(name="sbw", bufs=2) as sbw,
            tc.tile_pool(name="ps2", bufs=2, space="PSUM") as pp,
            tc.tile_pool(name="ps1", bufs=1, space="PSUM") as pp1,
        ):
            xg_loc = dpool.tile([NPC, F], dt.bfloat16)
            xg_full = dpool.tile([NG, F], dt.bfloat16, addr_space="Shared")
            part_loc = dpool.tile([1, F], dt.float32)
            pooled = dpool.tile([1, F], dt.float32, addr_space="Shared")

            # ---- persistent SBUF ----
            xT_sb = sbp.tile([128, NPC], dt.bfloat16)
            nc.sync.dma_start(xT_sb[:], xT)
            wt = {}
            for nm, apin, sh in (("Wgl", Wgl, [F, F]), ("bgl", bgl, [1, F]),
                                 ("Wgr", Wgr, [F, F]), ("bgr", bgr, [1, F])):
                tl = sbp.tile(sh, dt.bfloat16, tag=nm)
                nc.sync.dma_start(tl[:], apin)
                wt[nm] = tl
            ones_sb = sbp.tile([1, F], dt.bfloat16)
            nc.vector.memset(ones_sb[:], 1.0)
            ones_f = sbp.tile([128, 1], dt.float32)
            nc.vector.memset(ones_f[:], 1.0)

            iP_sb = sbp.tile([128, LW], dt.int16)
            nc.sync.dma_start(iP_sb[:], iP)
            par_sb = sbp.tile([128, TCpad], dt.float32)
            nc.sync.dma_start(par_sb[:], par)
            pm_sb = sbp.tile([128, TCpad], dt.float32)
            nc.sync.dma_start(pm_sb[:], pmask)
            sm_sb = sbp.tile([128, NB], dt.float32)
            nc.sync.dma_start(sm_sb[:], smask)
            smb_sb = sbp.tile([128, NB], dt.bfloat16)
            nc.sync.dma_start(smb_sb[:], smaskb)

            xgl_sb = sbp.tile([128, NB * F], dt.bfloat16)
            xgr_sb = sbp.tile([128, NB * F], dt.bfloat16)

            # ---- stage A: node tables (nodes in degree-sorted order) ----
            for ci in range(NB):
                lhs = xT_sb[:, 128 * ci:128 * (ci + 1)]
                for wn, bn_, dstt in (("Wgl", "bgl", xgl_sb),
                                      ("Wgr", "bgr", xgr_sb)):
                    ps = pp.tile([128, F], dt.float32, tag="psA")
                    nc.tensor.matmul(ps[:], lhsT=lhs, rhs=wt[wn][:],
                                     start=True, stop=False)
                    nc.tensor.matmul(ps[:], lhsT=ones_sb[:], rhs=wt[bn_][:],
                                     start=False, stop=True)
                    sl = dstt[:, F * ci:F * (ci + 1)]
                    nc.vector.tensor_copy(sl, ps[:])
                    if wn == "Wgl":
                        rows = slice(128 * ci, 128 * (ci + 1))
                        nc.sync.dma_start(xg_loc[rows, :], sl)

            nc.gpsimd.collective_compute(
                "AllGather", mybir.AluOpType.bypass, replica_groups=rg,
                ins=[xg_loc.opt()], outs=[xg_full.opt()])
            tab_pair = xg_full[:].rearrange("(a two) f -> a (two f)", two=2)

            # ---- main loop: one chunk = nb_c batches of equal D ----
            pacc = pp1.tile([F, 1], dt.float32, tag="pacc")
            first_mm = [True]
            nmm = sum(2 * nb * D + nb for (_, nb, D, _) in chunks)
            mmleft = [nmm]

            def acc_mm(lhsT, rhs):
                nc.tensor.matmul(pacc[:], lhsT=lhsT, rhs=rhs,
                                 start=first_mm[0],
                                 stop=(mmleft[0] == 1))
                first_mm[0] = False
                mmleft[0] -= 1

            for (b0, nb_c, D, coloff) in chunks:
                cols = nb_c * D
                nbF = nb_c * F
                bsl = slice(b0 * F, (b0 + nb_c) * F)
                # --- self-loop columns from local tables ---
                z0 = sbw.tile([128, NBC * F], dt.bfloat16, tag="z0")
                nc.vector.tensor_tensor(out=z0[:, :nbF], in0=xgl_sb[:, bsl],
                                        in1=xgr_sb[:, bsl], op=alu.add)
                za0 = sbw.tile([128, NBC * F], dt.bfloat16, tag="za0")
                nc.scalar.activation(za0[:, :nbF], z0[:, :nbF], act.Lrelu,
                                     alpha=0.2)
                m0 = za0[:, :nbF].rearrange("p (b f) -> p b f", f=F)
                l0p = sbw.tile([128, NBC], dt.float32, tag="l0p")
                l0n = sbw.tile([128, NBC], dt.float32, tag="l0n")
                nc.vector.tensor_reduce(l0p[:, :nb_c], m0[:, :, 0:PP],
                                        axis=mybir.AxisListType.X, op=alu.add)
                nc.vector.tensor_reduce(l0n[:, :nb_c], m0[:, :, PP:F],
                                        axis=mybir.AxisListType.X, op=alu.add)
                nc.vector.tensor_tensor(out=l0p[:, :nb_c], in0=l0p[:, :nb_c],
                                        in1=l0n[:, :nb_c], op=alu.subtract)
                E0 = sbw.tile([128, NBC], dt.float32, tag="E0")
                nc.scalar.activation(E0[:, :nb_c], l0p[:, :nb_c], act.Exp)

                if cols > 0:
                    csl = slice(coloff, coloff + cols)
                    gp = sbw.tile([128, CAPC * 2 * F], dt.bfloat16, tag="gp",
                                  bufs=3)
                    nc.gpsimd.dma_gather(
                        out_ap=gp[:, :cols * 2 * F].rearrange(
                            "p (c f) -> p c f", f=2 * F),
                        in_ap=tab_pair,
                        idxs_ap=iP_sb[:, coloff * 8:(coloff + cols) * 8],
                        num_idxs=cols * 128, num_idxs_reg=cols * 128,
                        elem_size=2 * F, single_packet=False)
                    # z = gp + xr[dst] on BOTH pair halves (DVE broadcast),
                    # then one contiguous Lrelu pass on ACT
                    z = sbw.tile([128, CAPC * 2 * F], dt.bfloat16, tag="z")
                    for bi in range(nb_c):
                        sl2 = slice(bi * D * 2 * F, (bi + 1) * D * 2 * F)
                        xr_b = xgr_sb[:, (b0 + bi) * F:(b0 + bi + 1) * F] \
                            .rearrange("p (one f) -> p one f", one=1) \
                            .to_broadcast((128, 2 * D, F))
                        nc.vector.tensor_tensor(
                            out=z[:, sl2].rearrange("p (c f) -> p c f", f=F),
                            in0=gp[:, sl2].rearrange("p (c f) -> p c f", f=F),
                            in1=xr_b, op=alu.add)
                    za = sbw.tile([128, CAPC * 2 * F], dt.bfloat16, tag="za")
                    nc.scalar.activation(za[:, :cols * 2 * F],
                                         z[:, :cols * 2 * F], act.Lrelu,
                                         alpha=0.2)
                    m3 = za[:, :cols * 2 * F].rearrange("p (c f) -> p c f",
                                                        f=2 * F)
                    lgA = sbw.tile([128, CAPC], dt.float32, tag="lgA")
                    lgn = sbw.tile([128, CAPC], dt.float32, tag="lgn")
                    lgB = sbw.tile([128, CAPC], dt.float32, tag="lgB")
                    lgn1 = sbw.tile([128, CAPC], dt.float32, tag="lgn1")
                    nc.vector.tensor_reduce(lgA[:, :cols], m3[:, :, 0:PP],
                                            axis=mybir.AxisListType.X,
                                            op=alu.add)
                    nc.vector.tensor_reduce(lgn[:, :cols], m3[:, :, PP:F],
                                            axis=mybir.AxisListType.X,
                                            op=alu.add)
                    nc.vector.tensor_reduce(lgB[:, :cols], m3[:, :, F:F + PP],
                                            axis=mybir.AxisListType.X,
                                            op=alu.add)
                    nc.vector.tensor_reduce(lgn1[:, :cols],
                                            m3[:, :, F + PP:2 * F],
                                            axis=mybir.AxisListType.X,
                                            op=alu.add)
                    # logit = A + par*(B-A),  A = lgA-lgn, B = lgB-lgn1
                    nc.vector.tensor_tensor(out=lgA[:, :cols],
                                            in0=lgA[:, :cols],
                                            in1=lgn[:, :cols], op=alu.subtract)
                    nc.vector.tensor_tensor(out=lgB[:, :cols],
                                            in0=lgB[:, :cols],
                                            in1=lgn1[:, :cols],
                                            op=alu.subtract)
                    nc.vector.tensor_tensor(out=lgB[:, :cols],
                                            in0=lgB[:, :cols],
                                            in1=lgA[:, :cols], op=alu.subtract)
                    nc.vector.tensor_tensor(out=lgB[:, :cols],
                                            in0=lgB[:, :cols],
                                            in1=par_sb[:, csl], op=alu.mult)
                    nc.vector.tensor_tensor(out=lgA[:, :cols],
                                            in0=lgA[:, :cols],
                                            in1=lgB[:, :cols], op=alu.add)
                    E = sbw.tile([128, CAPC], dt.float32, tag="E")
                    nc.scalar.activation(E[:, :cols], lgA[:, :cols], act.Exp)
                    nc.vector.tensor_tensor(out=E[:, :cols], in0=E[:, :cols],
                                            in1=pm_sb[:, csl], op=alu.mult)
                    den = sbw.tile([128, NBC], dt.float32, tag="den")
                    nc.vector.tensor_reduce(
                        den[:, :nb_c],
                        E[:, :cols].rearrange("p (b d) -> p b d", d=D),
                        axis=mybir.AxisListType.X, op=alu.add)
                    nc.vector.tensor_tensor(out=den[:, :nb_c],
                                            in0=den[:, :nb_c],
                                            in1=E0[:, :nb_c], op=alu.add)
                else:
                    den = E0

                rd = sbw.tile([128, NBC], dt.float32, tag="rd")
                nc.vector.reciprocal(rd[:, :nb_c], den[:, :nb_c])
                # w0 = E0 * rd * smask  (bf16 for the PE)
                w0 = sbw.tile([128, NBC], dt.float32, tag="w0")
                nc.vector.tensor_tensor(out=w0[:, :nb_c], in0=E0[:, :nb_c],
                                        in1=rd[:, :nb_c], op=alu.mult)
                nc.vector.tensor_tensor(out=w0[:, :nb_c], in0=w0[:, :nb_c],
                                        in1=sm_sb[:, b0:b0 + nb_c],
                                        op=alu.mult)
                w0b = sbw.tile([128, NBC], dt.bfloat16, tag="w0b", bufs=3)
                nc.vector.tensor_copy(w0b[:, :nb_c], w0[:, :nb_c])
                if cols > 0:
                    w = sbw.tile([128, CAPC], dt.float32, tag="w")
                    rd_b = rd[:, :nb_c].rearrange(
                        "p (b one) -> p b one", one=1).to_broadcast(
                        (128, nb_c, D))
                    nc.vector.tensor_tensor(
                        out=w[:, :cols].rearrange("p (b d) -> p b d", d=D),
                        in0=E[:, :cols].rearrange("p (b d) -> p b d", d=D),
                        in1=rd_b, op=alu.mult)
                    # pair-half weights: w1 = w*par ; w0h = w - w1
                    w1 = sbw.tile([128, CAPC], dt.float32, tag="w1")
                    nc.vector.tensor_tensor(out=w1[:, :cols], in0=w[:, :cols],
                                            in1=par_sb[:, csl], op=alu.mult)
                    nc.vector.tensor_tensor(out=w[:, :cols], in0=w[:, :cols],
                                            in1=w1[:, :cols], op=alu.subtract)
                    wLb = sbw.tile([128, CAPC], dt.bfloat16, tag="wLb", bufs=3)
                    wHb = sbw.tile([128, CAPC], dt.bfloat16, tag="wHb", bufs=3)
                    nc.vector.tensor_copy(wLb[:, :cols], w[:, :cols])
                    nc.vector.tensor_copy(wHb[:, :cols], w1[:, :cols])
                    # weighted sum directly on the RAW gathered xl rows
                    for t in range(cols):
                        acc_mm(gp[:, t * 2 * F:t * 2 * F + F],
                               wLb[:, t:t + 1])
                        acc_mm(gp[:, t * 2 * F + F:(t + 1) * 2 * F],
                               wHb[:, t:t + 1])
                for bi in range(nb_c):
                    acc_mm(xgl_sb[:, (b0 + bi) * F:(b0 + bi + 1) * F],
                           w0b[:, bi:bi + 1])

            # ---- tail:  S = pacc ; AllReduce ; head ----
            part_sb = sbp.tile([F, 1], dt.float32)
            nc.vector.tensor_copy(part_sb[:], pacc[:])
            nc.sync.dma_start(part_loc[:].rearrange("o f -> f o"), part_sb[:])

            nc.gpsimd.collective_compute(
                "AllReduce", alu.add, replica_groups=rg,
                ins=[part_loc.opt()], outs=[pooled.opt()])

            # ---- head ----
            pool_sb = sbp.tile([F, 1], dt.float32)
            nc.sync.dma_start(pool_sb[:], pooled[:].rearrange("o f -> f o"))
            Ap_sb = sbp.tile([F, 1], dt.float32)
            nc.sync.dma_start(Ap_sb[:], Ap)
            Bp_sb = sbp.tile([F, 1], dt.float32)
            nc.sync.dma_start(Bp_sb[:], Bp)
            Wc_sb = sbp.tile([F, NCLS], dt.float32)
            nc.sync.dma_start(Wc_sb[:], Wcp)
            bc_sb = sbp.tile([1, NCLS], dt.float32)
            nc.sync.dma_start(bc_sb[:], bc)
            h_sb = sbp.tile([F, 1], dt.float32)
            nc.vector.scalar_tensor_tensor(
                out=h_sb[:], in0=pool_sb[:], scalar=Ap_sb[:, 0:1], in1=Bp_sb[:],
                op0=alu.mult, op1=alu.add)
            one1 = sbp.tile([1, 1], dt.float32)
            nc.vector.memset(one1[:], 1.0)
            hp = pp1.tile([1, NCLS], dt.float32, tag="hp")
            nc.tensor.matmul(hp[:], lhsT=h_sb[:], rhs=Wc_sb[:], start=True,
                             stop=False)
            nc.tensor.matmul(hp[:], lhsT=one1[:], rhs=bc_sb[:], start=False,
                             stop=True)
            eh = sbp.tile([1, NCLS], dt.float32)
            nc.scalar.activation(eh[:], hp[:], act.Exp)
            den = sbp.tile([1, 1], dt.float32)
            nc.vector.tensor_reduce(den[:], eh[:], axis=mybir.AxisListType.X,
                                    op=alu.add)
            rden = sbp.tile([1, 1], dt.float32)
            nc.vector.reciprocal(rden[:], den[:])
            osb = sbp.tile([1, NCLS], dt.float32)
            nc.vector.tensor_scalar(out=osb[:], in0=eh[:], scalar1=rden[:, 0:1],
                                    scalar2=None, op0=alu.mult)
            nc.sync.dma_start(out, osb[:])

    nc.compile()
    return nc


# --------------------------------------------------------------------------
# public entry point
# --------------------------------------------------------------------------

_CACHE = {}


def _install_ntff_hook():
    """Provide antenv.axon_hooks + the ctypes NTFF hook when the image lacks
    them, so run_bass_kernel_spmd(trace=True) can capture exec_time_ns."""
    import contextlib
    import ctypes
    import sys
    import types

    try:
        import antenv.axon_hooks  # noqa: F401
        return
    except ImportError:
        pass
    try:
        import antenv
    except ImportError:
        return
    holder = [None]
    mod = types.ModuleType("antenv.axon_hooks")
    mod.set_axon_ntff_profile_hook = lambda h: holder.__setitem__(0, h)
    mod.get_axon_ntff_profile_hook = lambda: holder[0]
    sys.modules["antenv.axon_hooks"] = mod
    antenv.axon_hooks = mod

    so_path = "/opt/axon/libaxon_pjrt.so"
    if os.path.exists(so_path):
        lib = ctypes.CDLL(so_path)
        if hasattr(lib, "axon_start_nrt_profile"):
            lib.axon_start_nrt_profile.argtypes = [
                ctypes.POINTER(ctypes.c_int64), ctypes.c_size_t]
            lib.axon_start_nrt_profile.restype = ctypes.c_int64
            lib.axon_stop_nrt_profile.argtypes = [ctypes.c_char_p]
            lib.axon_stop_nrt_profile.restype = ctypes.c_int64

            @contextlib.contextmanager
            def _hook(output_dir, device_ids):
                import jax
                jax.devices()
                if device_ids:
                    ids = (ctypes.c_int64 * len(device_ids))(*device_ids)
                    rc = lib.axon_start_nrt_profile(ids, len(device_ids))
                else:
                    rc = lib.axon_start_nrt_profile(None, 0)
                if rc != 0:
                    raise RuntimeError(f"axon_start_nrt_profile rc={rc}")
                try:
                    yield
                finally:
                    n = lib.axon_stop_nrt_profile(str(output_dir).encode())
                    print(f"ntff profile: {n} file(s) -> {output_dir}")

            mod.set_axon_ntff_profile_hook(_hook)

    import concourse.bass_utils as bu
    bu.upload_artifacts = lambda tmpdir: "local://" + str(tmpdir)


def kernel(**inputs):
    from concourse.bass_utils import run_bass_kernel_spmd

    if bool(int(os.environ.get("KERNEL_TRACE", "0"))):
        _install_ntff_hook()
    inputs = {k: np.asarray(v) for k, v in inputs.items()}
    in_maps, meta = prep_host(**inputs)
    key = (meta["PP"], meta["TC"], meta["chunks"])
    if key not in _CACHE:
        _CACHE[key] = build(meta)
    nc = _CACHE[key]
    res = run_bass_kernel_spmd(nc, in_maps, core_ids=list(range(M)),
                               trace=bool(int(os.environ.get("KERNEL_TRACE", "0"))))
    if getattr(res, "exec_time_ns", None) is not None:
        print(f"HW exec time: {res.exec_time_ns} ns")
    return np.asarray(res.results[0]["out"]).astype(np.float32)


# revision 16
# speedup vs baseline: 1.3670x; 1.3670x over previous
"""Distributed Trainium2 kernel for a GATv2 layer + BN + global-mean-pool + classifier.

Math (reference, heads=1):
    xl = x@Wl + bl ; xr = x@Wr + br
    logit_e = att . leaky_relu(xl[src_e] + xr[dst_e], 0.2)
    a_e     = segment_softmax(logit_e over dst)
    out_i   = sum_{e: dst=i} a_e * xl[src_e] ; out = out + bias1
    h       = BN(out) ; g = mean_i h ; y = softmax(g@Wc + bc)

Only the global mean over nodes matters, so per-node outputs never
materialize:
    y = softmax( ((S/N)*A + B) @ Wc + bc ),  S = sum_e a_e * xl[src_e],
    A = gamma/sqrt(var+eps), B = (bias1 - mu)*A + beta.

Attention weights v = att are folded into the tables host-side:
    v_f * lrelu(z_f) = sign_f * lrelu(|v_f| z_f)
with features permuted so positive-sign features occupy columns [0,PP).

Layout: per core, nodes sorted by in-degree (desc) and processed 128 per
batch, one node per partition; a node's in-edges occupy D consecutive
slot-columns of its partition row (D = max degree in batch; batches with
equal D are grouped into chunks).  Per edge ONE dma_gather fetches the
packed pair row of xl[src] (int16 indices address node pairs); xr[dst] is
a stride-0 broadcast of the node's own row; the self-loop edge is computed
straight from the local tables (no gather).  Segment softmax is a plain
row-reduce per batch.  The weighted sum uses sum_e w_e z_e - sum_d xr_d
(softmax weights sum to 1 per node), accumulated per-column into a
[128, F] accumulator, finished with one ones-matmul + AllReduce + head.
"""

import os

import ml_dtypes
import numpy as np

import concourse.bass as bass
import concourse.bacc as bacc
import concourse.mybir as mybir
import concourse.tile as tile

M = 8  # cores
F = 128
NCLS = 5
BN_EPS = 1e-5
NPCR = 6250     # real nodes per core
NB = 49         # batches of 128 nodes (6272 padded)
NPC = NB * 128
NG = M * NPC
CAP = 32        # max slot-columns per chunk (SBUF budget)
NBC = 16        # max batches per chunk

BF16 = ml_dtypes.bfloat16


def _wrap_idx(seq):
    """[n] int array -> [128, n//16] int16 wrap layout (16-partition groups,
    replicated across the 8 gpsimd cores)."""
    n = seq.shape[0]
    assert n % 16 == 0
    w = seq.reshape(n // 16, 16).T.astype(np.int16)
    return np.tile(w, (8, 1))


def prep_host(x, edge_index, Wl, bl, Wr, br, att, bias1,
              bn_gamma, bn_beta, bn_mean, bn_var, Wc, bc):
    N = x.shape[0]
    assert N == NPCR * M
    src = np.asarray(edge_index[0], np.int64)
    dst = np.asarray(edge_index[1], np.int64)

    # ---- attention folding ----
    v = np.asarray(att[0], np.float64)
    posm = v >= 0
    perm = np.argsort(~posm, kind="stable")
    PP = int(posm.sum())
    assert 0 < PP < F, f"degenerate attention sign split PP={PP}"
    absv = np.abs(v[perm])
    Wg_l = (Wl[:, perm] * absv[None, :]).astype(np.float32)
    bg_l = (bl[perm] * absv).astype(np.float32)
    Wg_r = (Wr[:, perm] * absv[None, :]).astype(np.float32)
    bg_r = (br[perm] * absv).astype(np.float32)

    # ---- per-core degree-sorted node order ----
    deg = np.bincount(dst, minlength=N)  # in-degree excluding self loop
    rank = np.zeros(N, np.int64)         # node -> global padded rank
    xT = np.zeros((M, 128, NPC), BF16)
    smask = np.zeros((M, 128, NB), np.float32)  # real-node mask [p, b]
    Dbs = np.zeros((M, NB), np.int64)
    for k in range(M):
        lo = k * NPCR
        dk = deg[lo:lo + NPCR]
        order = np.argsort(-dk, kind="stable")     # rank -> local node
        rank[lo + order] = k * NPC + np.arange(NPCR)
        xk = np.zeros((NPC, F), np.float32)
        xk[:NPCR] = x[lo + order]
        xT[k] = np.ascontiguousarray(xk.T.astype(BF16))
        r = np.arange(NPC)
        smask[k] = ((r % 128) * 0 + (r < NPCR)).astype(np.float32) \
            .reshape(NB, 128).T
        Dbs[k] = np.concatenate([np.sort(dk)[::-1], np.zeros(NPC - NPCR,
                                np.int64)]).reshape(NB, 128).max(axis=1)

    # per-core chunk schedules must be IDENTICAL (SPMD one program).
    # Use the max D over cores for each batch index.
    Dmax_b = Dbs.max(axis=0)           # [NB] non-increasing? per-core sorted
    Dmax_b = np.maximum.accumulate(Dmax_b[::-1])[::-1]  # enforce non-increasing
    chunks = []   # (b0, nb_c, D, coloff)
    coloff = 0
    b = 0
    while b < NB:
        D = int(Dmax_b[b])
        e = b
        while e < NB and int(Dmax_b[e]) == D:
            e += 1
        run = e - b
        step = max(1, min(NBC, (CAP // D) if D > 0 else NBC))
        while b < e:
            nb_c = min(step, e - b)
            chunks.append((b, nb_c, D, coloff))
            coloff += nb_c * D
            b += nb_c
    TC = coloff  # total gathered columns
    TCpad = ((TC + 1 + 7) // 8) * 8  # pad idx width to mult of 8 cols

    # ---- per-core slot tables ----
    # CSR of in-edges by dst, in rank order
    iP = np.zeros((M, 128, TCpad), np.int64)
    par = np.zeros((M, 128, TCpad), np.float32)
    pmask = np.zeros((M, 128, TCpad), np.float32)
    srcrow = rank[src]
    for k in range(M):
        lo = k * NPCR
        sel = (dst >= lo) & (dst < lo + NPCR)
        d_r = rank[dst[sel]] - k * NPC        # local rank of dst
        s_r = srcrow[sel]                     # global padded rank of src
        o = np.argsort(d_r, kind="stable")
        d_r = d_r[o]
        s_r = s_r[o]
        cnt = np.bincount(d_r, minlength=NPC)
        starts = np.concatenate([[0], np.cumsum(cnt)])
        # slot (p, col) for chunk (b0, nb, D): col = coloff + bi*D + d
        # edge d of node rank (b0+bi)*128 + p
        pos_in_seg = np.arange(len(d_r)) - starts[d_r]
        bnode = d_r // 128
        pnode = d_r % 128
        # find chunk of bnode
        colbase = np.zeros(NB, np.int64)
        Dof = np.zeros(NB, np.int64)
        for (b0, nb_c, D, co) in chunks:
            for bi in range(nb_c):
                colbase[b0 + bi] = co + bi * D
                Dof[b0 + bi] = D
        assert (pos_in_seg < Dof[bnode]).all()
        cols = colbase[bnode] + pos_in_seg
        iP[k, pnode, cols] = s_r >> 1
        par[k, pnode, cols] = (s_r & 1).astype(np.float32)
        pmask[k, pnode, cols] = 1.0

    iP_w = np.stack([
        _wrap_idx(iP[k, :, :TCpad].T.reshape(-1)) for k in range(M)])

    # ---- head constants ----
    A = bn_gamma.astype(np.float64) / np.sqrt(bn_var.astype(np.float64) + BN_EPS)
    Ap = (A[perm] / (N * absv)).astype(np.float32).reshape(F, 1)
    Bp = ((bias1 - bn_mean).astype(np.float64) * A + bn_beta)[perm] \
        .astype(np.float32).reshape(F, 1)
    Wcp = Wc[perm, :].astype(np.float32)

    meta = dict(PP=PP, TC=TC, TCpad=TCpad, chunks=tuple(chunks))

    in_maps = []
    for k in range(M):
        in_maps.append({
            "xT": np.ascontiguousarray(xT[k]),
            "Wgl": Wg_l.astype(BF16),
            "bgl": bg_l.reshape(1, F).astype(BF16),
            "Wgr": Wg_r.astype(BF16),
            "bgr": bg_r.reshape(1, F).astype(BF16),
            "iP": np.ascontiguousarray(iP_w[k]),
            "par": np.ascontiguousarray(par[k]),
            "pmask": np.ascontiguousarray(pmask[k]),
            "smask": np.ascontiguousarray(smask[k]),
            "smaskb": np.ascontiguousarray(smask[k].astype(BF16)),
            "Ap": Ap,
            "Bp": Bp,
            "Wcp": Wcp,
            "bc": bc.reshape(1, NCLS).astype(np.float32),
        })
    return in_maps, meta


def build(meta):
    PP, TC, TCpad = meta["PP"], meta["TC"], meta["TCpad"]
    chunks = meta["chunks"]
    CAPC = max(CAP, max(nb * D for (_, nb, D, _) in chunks))
    LW = (TCpad * 128) // 16

    dt = mybir.dt
    alu = mybir.AluOpType
    act = mybir.ActivationFunctionType
    rg = [list(range(M))]

    nc = bacc.Bacc("TRN2", target_bir_lowering=False, debug=False, num_devices=M)

    def p_in(name, shape, d):
        return nc.dram_tensor(name, shape, d, kind="ExternalInput").ap()

    xT = p_in("xT", [128, NPC], dt.bfloat16)
    Wgl = p_in("Wgl", [F, F], dt.bfloat16)
    bgl = p_in("bgl", [1, F], dt.bfloat16)
    Wgr = p_in("Wgr", [F, F], dt.bfloat16)
    bgr = p_in("bgr", [1, F], dt.bfloat16)
    iP = p_in("iP", [128, LW], dt.int16)
    par = p_in("par", [128, TCpad], dt.float32)
    pmask = p_in("pmask", [128, TCpad], dt.float32)
    smask = p_in("smask", [128, NB], dt.float32)
    smaskb = p_in("smaskb", [128, NB], dt.bfloat16)
    Ap = p_in("Ap", [F, 1], dt.float32)
    Bp = p_in("Bp", [F, 1], dt.float32)
    Wcp = p_in("Wcp", [F, NCLS], dt.float32)
    bc = p_in("bc", [1, NCLS], dt.float32)
    out = nc.dram_tensor("out", [1, NCLS], dt.float32, kind="ExternalOutput").ap()

    with tile.TileContext(nc) as tc:
        with (
            tc.tile_pool(name="dram", bufs=1, space="DRAM") as dpool,
            tc.tile_pool(name="sbp", bufs=1) as sbp,
            tc.tile_pool(name="sbw", bufs=2) as sbw,
            tc.tile_pool(name="ps2", bufs=2, space="PSUM") as pp,
            tc.tile_pool(name="ps1", bufs=1, space="PSUM") as pp1,
        ):
            xg_loc = dpool.tile([NPC, F], dt.bfloat16)
            xg_full = dpool.tile([NG, F], dt.bfloat16, addr_space="Shared")
            part_loc = dpool.tile([1, F], dt.float32)
            pooled = dpool.tile([1, F], dt.float32, addr_space="Shared")

            # ---- persistent SBUF ----
            xT_sb = sbp.tile([128, NPC], dt.bfloat16)
            nc.sync.dma_start(xT_sb[:], xT)
            wt = {}
            for nm, apin, sh in (("Wgl", Wgl, [F, F]), ("bgl", bgl, [1, F]),
                                 ("Wgr", Wgr, [F, F]), ("bgr", bgr, [1, F])):
                tl = sbp.tile(sh, dt.bfloat16, tag=nm)
                nc.sync.dma_start(tl[:], apin)
                wt[nm] = tl
            ones_sb = sbp.tile([1, F], dt.bfloat16)
            nc.vector.memset(ones_sb[:], 1.0)
            ones_f = sbp.tile([128, 1], dt.float32)
            nc.vector.memset(ones_f[:], 1.0)

            iP_sb = sbp.tile([128, LW], dt.int16)
            nc.sync.dma_start(iP_sb[:], iP)
            par_sb = sbp.tile([128, TCpad], dt.float32)
            nc.sync.dma_start(par_sb[:], par)
            pm_sb = sbp.tile([128, TCpad], dt.float32)
            nc.sync.dma_start(pm_sb[:], pmask)
            sm_sb = sbp.tile([128, NB], dt.float32)
            nc.sync.dma_start(sm_sb[:], smask)
            smb_sb = sbp.tile([128, NB], dt.bfloat16)
            nc.sync.dma_start(smb_sb[:], smaskb)

            xgl_sb = sbp.tile([128, NB * F], dt.bfloat16)
            xgr_sb = sbp.tile([128, NB * F], dt.bfloat16)

            # ---- stage A: node tables (nodes in degree-sorted order) ----
            for ci in range(NB):
                lhs = xT_sb[:, 128 * ci:128 * (ci + 1)]
                for wn, bn_, dstt in (("Wgl", "bgl", xgl_sb),
                                      ("Wgr", "bgr", xgr_sb)):
                    ps = pp.tile([128, F], dt.float32, tag="psA")
                    nc.tensor.matmul(ps[:], lhsT=lhs, rhs=wt[wn][:],
                                     start=True, stop=False)
                    nc.tensor.matmul(ps[:], lhsT=ones_sb[:], rhs=wt[bn_][:],
                                     start=False, stop=True)
                    sl = dstt[:, F * ci:F * (ci + 1)]
                    nc.vector.tensor_copy(sl, ps[:])
                    if wn == "Wgl":
                        rows = slice(128 * ci, 128 * (ci + 1))
                        nc.sync.dma_start(xg_loc[rows, :], sl)

            nc.gpsimd.collective_compute(
                "AllGather", mybir.AluOpType.bypass, replica_groups=rg,
                ins=[xg_loc.opt()], outs=[xg_full.opt()])
            tab_pair = xg_full[:].rearrange("(a two) f -> a (two f)", two=2)

            # ---- main loop: one chunk = nb_c batches of equal D ----
            pacc = pp1.tile([F, 1], dt.float32, tag="pacc")
            first_mm = [True]
            nmm = sum(2 * nb * D + nb for (_, nb, D, _) in chunks)
            mmleft = [nmm]

            def acc_mm(lhsT, rhs):
                nc.tensor.matmul(pacc[:], lhsT=lhsT, rhs=rhs,
                                 start=first_mm[0],
                                 stop=(mmleft[0] == 1))
                first_mm[0] = False
                mmleft[0] -= 1

            for (b0, nb_c, D, coloff) in chunks:
                cols = nb_c * D
                nbF = nb_c * F
                bsl = slice(b0 * F, (b0 + nb_c) * F)
                # --- self-loop columns from local tables ---
                z0 = sbw.tile([128, NBC * F], dt.bfloat16, tag="z0")
                nc.vector.tensor_tensor(out=z0[:, :nbF], in0=xgl_sb[:, bsl],
                                        in1=xgr_sb[:, bsl], op=alu.add)
                nc.scalar.activation(z0[:, :nbF], z0[:, :nbF], act.Lrelu,
                                     alpha=0.2)
                m0 = z0[:, :nbF].rearrange("p (b f) -> p b f", f=F)
                l0p = sbw.tile([128, NBC], dt.float32, tag="l0p")
                l0n = sbw.tile([128, NBC], dt.float32, tag="l0n")
                nc.vector.tensor_reduce(l0p[:, :nb_c], m0[:, :, 0:PP],
                                        axis=mybir.AxisListType.X, op=alu.add)
                nc.vector.tensor_reduce(l0n[:, :nb_c], m0[:, :, PP:F],
                                        axis=mybir.AxisListType.X, op=alu.add)
                nc.vector.tensor_tensor(out=l0p[:, :nb_c], in0=l0p[:, :nb_c],
                                        in1=l0n[:, :nb_c], op=alu.subtract)
                E0 = sbw.tile([128, NBC], dt.float32, tag="E0")
                nc.scalar.activation(E0[:, :nb_c], l0p[:, :nb_c], act.Exp)

                if cols > 0:
                    csl = slice(coloff, coloff + cols)
                    gp = sbw.tile([128, CAPC * 2 * F], dt.bfloat16, tag="gp",
                                  bufs=4)
                    nc.gpsimd.dma_gather(
                        out_ap=gp[:, :cols * 2 * F].rearrange(
                            "p (c f) -> p c f", f=2 * F),
                        in_ap=tab_pair,
                        idxs_ap=iP_sb[:, coloff * 8:(coloff + cols) * 8],
                        num_idxs=cols * 128, num_idxs_reg=cols * 128,
                        elem_size=2 * F, single_packet=False)
                    # z = gp + xr[dst] on BOTH pair halves (DVE broadcast),
                    # then one contiguous Lrelu pass on ACT
                    z = sbw.tile([128, CAPC * 2 * F], dt.bfloat16, tag="z")
                    for bi in range(nb_c):
                        sl2 = slice(bi * D * 2 * F, (bi + 1) * D * 2 * F)
                        xr_b = xgr_sb[:, (b0 + bi) * F:(b0 + bi + 1) * F] \
                            .rearrange("p (one f) -> p one f", one=1) \
                            .to_broadcast((128, 2 * D, F))
                        nc.vector.tensor_tensor(
                            out=z[:, sl2].rearrange("p (c f) -> p c f", f=F),
                            in0=gp[:, sl2].rearrange("p (c f) -> p c f", f=F),
                            in1=xr_b, op=alu.add)
                    nc.scalar.activation(z[:, :cols * 2 * F],
                                         z[:, :cols * 2 * F], act.Lrelu,
                                         alpha=0.2)
                    m3 = z[:, :cols * 2 * F].rearrange("p (c f) -> p c f",
                                                       f=2 * F)
                    lgA = sbw.tile([128, CAPC], dt.float32, tag="lgA")
                    lgn = sbw.tile([128, CAPC], dt.float32, tag="lgn")
                    lgB = sbw.tile([128, CAPC], dt.float32, tag="lgB")
                    lgn1 = sbw.tile([128, CAPC], dt.float32, tag="lgn1")
                    nc.vector.tensor_reduce(lgA[:, :cols], m3[:, :, 0:PP],
                                            axis=mybir.AxisListType.X,
                                            op=alu.add)
                    nc.vector.tensor_reduce(lgn[:, :cols], m3[:, :, PP:F],
                                            axis=mybir.AxisListType.X,
                                            op=alu.add)
                    nc.vector.tensor_reduce(lgB[:, :cols], m3[:, :, F:F + PP],
                                            axis=mybir.AxisListType.X,
                                            op=alu.add)
                    nc.vector.tensor_reduce(lgn1[:, :cols],
                                            m3[:, :, F + PP:2 * F],
                                            axis=mybir.AxisListType.X,
                                            op=alu.add)
                    # logit = A + par*(B-A),  A = lgA-lgn, B = lgB-lgn1
                    nc.vector.tensor_tensor(out=lgA[:, :cols],
                                            in0=lgA[:, :cols],
                                            in1=lgn[:, :cols], op=alu.subtract)
                    nc.vector.tensor_tensor(out=lgB[:, :cols],
                                            in0=lgB[:, :cols],
                                            in1=lgn1[:, :cols],
                                            op=alu.subtract)
                    nc.vector.tensor_tensor(out=lgB[:, :cols],
                                            in0=lgB[:, :cols],
                                            in1=lgA[:, :cols], op=alu.subtract)
                    nc.vector.tensor_tensor(out=lgB[:, :cols],
                                            in0=lgB[:, :cols],
                                            in1=par_sb[:, csl], op=alu.mult)
                    nc.vector.tensor_tensor(out=lgA[:, :cols],
                                            in0=lgA[:, :cols],
                                            in1=lgB[:, :cols], op=alu.add)
                    E = sbw.tile([128, CAPC], dt.float32, tag="E")
                    nc.scalar.activation(E[:, :cols], lgA[:, :cols], act.Exp)
                    nc.vector.tensor_tensor(out=E[:, :cols], in0=E[:, :cols],
                                            in1=pm_sb[:, csl], op=alu.mult)
                    den = sbw.tile([128, NBC], dt.float32, tag="den")
                    nc.vector.tensor_reduce(
                        den[:, :nb_c],
                        E[:, :cols].rearrange("p (b d) -> p b d", d=D),
                        axis=mybir.AxisListType.X, op=alu.add)
                    nc.vector.tensor_tensor(out=den[:, :nb_c],
                                            in0=den[:, :nb_c],
                                            in1=E0[:, :nb_c], op=alu.add)
                else:
                    den = E0

                rd = sbw.tile([128, NBC], dt.float32, tag="rd")
                nc.vector.reciprocal(rd[:, :nb_c], den[:, :nb_c])
                # w0 = E0 * rd * smask  (bf16 for the PE)
                w0 = sbw.tile([128, NBC], dt.float32, tag="w0")
                nc.vector.tensor_tensor(out=w0[:, :nb_c], in0=E0[:, :nb_c],
                                        in1=rd[:, :nb_c], op=alu.mult)
                nc.vector.tensor_tensor(out=w0[:, :nb_c], in0=w0[:, :nb_c],
                                        in1=sm_sb[:, b0:b0 + nb_c],
                                        op=alu.mult)
                w0b = sbw.tile([128, NBC], dt.bfloat16, tag="w0b", bufs=3)
                nc.vector.tensor_copy(w0b[:, :nb_c], w0[:, :nb_c])
                if cols > 0:
                    w = sbw.tile([128, CAPC], dt.float32, tag="w")
                    rd_b = rd[:, :nb_c].rearrange(
                        "p (b one) -> p b one", one=1).to_broadcast(
                        (128, nb_c, D))
                    nc.vector.tensor_tensor(
                        out=w[:, :cols].rearrange("p (b d) -> p b d", d=D),
                        in0=E[:, :cols].rearrange("p (b d) -> p b d", d=D),
                        in1=rd_b, op=alu.mult)
                    # pair-half weights: w1 = w*par ; w0h = w - w1
                    w1 = sbw.tile([128, CAPC], dt.float32, tag="w1")
                    nc.vector.tensor_tensor(out=w1[:, :cols], in0=w[:, :cols],
                                            in1=par_sb[:, csl], op=alu.mult)
                    nc.vector.tensor_tensor(out=w[:, :cols], in0=w[:, :cols],
                                            in1=w1[:, :cols], op=alu.subtract)
                    wLb = sbw.tile([128, CAPC], dt.bfloat16, tag="wLb", bufs=3)
                    wHb = sbw.tile([128, CAPC], dt.bfloat16, tag="wHb", bufs=3)
                    nc.vector.tensor_copy(wLb[:, :cols], w[:, :cols])
                    nc.vector.tensor_copy(wHb[:, :cols], w1[:, :cols])
                    # weighted sum directly on the RAW gathered xl rows
                    for t in range(cols):
                        acc_mm(gp[:, t * 2 * F:t * 2 * F + F],
                               wLb[:, t:t + 1])
                        acc_mm(gp[:, t * 2 * F + F:(t + 1) * 2 * F],
                               wHb[:, t:t + 1])
                for bi in range(nb_c):
                    acc_mm(xgl_sb[:, (b0 + bi) * F:(b0 + bi + 1) * F],
                           w0b[:, bi:bi + 1])

            # ---- tail:  S = pacc ; AllReduce ; head ----
            part_sb = sbp.tile([F, 1], dt.float32)
            nc.vector.tensor_copy(part_sb[:], pacc[:])
            nc.sync.dma_start(part_loc[:].rearrange("o f -> f o"), part_sb[:])

            nc.gpsimd.collective_compute(
                "AllReduce", alu.add, replica_groups=rg,
                ins=[part_loc.opt()], outs=[pooled.opt()])

            # ---- head ----
            pool_sb = sbp.tile([F, 1], dt.float32)
            nc.sync.dma_start(pool_sb[:], pooled[:].rearrange("o f -> f o"))
            Ap_sb = sbp.tile([F, 1], dt.float32)
            nc.sync.dma_start(Ap_sb[:], Ap)
            Bp_sb = sbp.tile([F, 1], dt.float32)
            nc.sync.dma_start(Bp_sb[:], Bp)
            Wc_sb = sbp.tile([F, NCLS], dt.float32)
            nc.sync.dma_start(Wc_sb[:], Wcp)
            bc_sb = sbp.tile([1, NCLS], dt.float32)
            nc.sync.dma_start(bc_sb[:], bc)
            h_sb = sbp.tile([F, 1], dt.float32)
            nc.vector.scalar_tensor_tensor(
                out=h_sb[:], in0=pool_sb[:], scalar=Ap_sb[:, 0:1], in1=Bp_sb[:],
                op0=alu.mult, op1=alu.add)
            one1 = sbp.tile([1, 1], dt.float32)
            nc.vector.memset(one1[:], 1.0)
            hp = pp1.tile([1, NCLS], dt.float32, tag="hp")
            nc.tensor.matmul(hp[:], lhsT=h_sb[:], rhs=Wc_sb[:], start=True,
                             stop=False)
            nc.tensor.matmul(hp[:], lhsT=one1[:], rhs=bc_sb[:], start=False,
                             stop=True)
            eh = sbp.tile([1, NCLS], dt.float32)
            nc.scalar.activation(eh[:], hp[:], act.Exp)
            den = sbp.tile([1, 1], dt.float32)
            nc.vector.tensor_reduce(den[:], eh[:], axis=mybir.AxisListType.X,
                                    op=alu.add)
            rden = sbp.tile([1, 1], dt.float32)
            nc.vector.reciprocal(rden[:], den[:])
            osb = sbp.tile([1, NCLS], dt.float32)
            nc.vector.tensor_scalar(out=osb[:], in0=eh[:], scalar1=rden[:, 0:1],
                                    scalar2=None, op0=alu.mult)
            nc.sync.dma_start(out, osb[:])

    nc.compile()
    return nc


# --------------------------------------------------------------------------
# public entry point
# --------------------------------------------------------------------------

_CACHE = {}


def _install_ntff_hook():
    """Provide antenv.axon_hooks + the ctypes NTFF hook when the image lacks
    them, so run_bass_kernel_spmd(trace=True) can capture exec_time_ns."""
    import contextlib
    import ctypes
    import sys
    import types

    try:
        import antenv.axon_hooks  # noqa: F401
        return
    except ImportError:
        pass
    try:
        import antenv
    except ImportError:
        return
    holder = [None]
    mod = types.ModuleType("antenv.axon_hooks")
    mod.set_axon_ntff_profile_hook = lambda h: holder.__setitem__(0, h)
    mod.get_axon_ntff_profile_hook = lambda: holder[0]
    sys.modules["antenv.axon_hooks"] = mod
    antenv.axon_hooks = mod

    so_path = "/opt/axon/libaxon_pjrt.so"
    if os.path.exists(so_path):
        lib = ctypes.CDLL(so_path)
        if hasattr(lib, "axon_start_nrt_profile"):
            lib.axon_start_nrt_profile.argtypes = [
                ctypes.POINTER(ctypes.c_int64), ctypes.c_size_t]
            lib.axon_start_nrt_profile.restype = ctypes.c_int64
            lib.axon_stop_nrt_profile.argtypes = [ctypes.c_char_p]
            lib.axon_stop_nrt_profile.restype = ctypes.c_int64

            @contextlib.contextmanager
            def _hook(output_dir, device_ids):
                import jax
                jax.devices()
                if device_ids:
                    ids = (ctypes.c_int64 * len(device_ids))(*device_ids)
                    rc = lib.axon_start_nrt_profile(ids, len(device_ids))
                else:
                    rc = lib.axon_start_nrt_profile(None, 0)
                if rc != 0:
                    raise RuntimeError(f"axon_start_nrt_profile rc={rc}")
                try:
                    yield
                finally:
                    n = lib.axon_stop_nrt_profile(str(output_dir).encode())
                    print(f"ntff profile: {n} file(s) -> {output_dir}")

            mod.set_axon_ntff_profile_hook(_hook)

    import concourse.bass_utils as bu
    bu.upload_artifacts = lambda tmpdir: "local://" + str(tmpdir)


def kernel(**inputs):
    from concourse.bass_utils import run_bass_kernel_spmd

    if bool(int(os.environ.get("KERNEL_TRACE", "0"))):
        _install_ntff_hook()
    inputs = {k: np.asarray(v) for k, v in inputs.items()}
    in_maps, meta = prep_host(**inputs)
    key = (meta["PP"], meta["TC"], meta["chunks"])
    if key not in _CACHE:
        _CACHE[key] = build(meta)
    nc = _CACHE[key]
    res = run_bass_kernel_spmd(nc, in_maps, core_ids=list(range(M)),
                               trace=bool(int(os.environ.get("KERNEL_TRACE", "0"))))
    if getattr(res, "exec_time_ns", None) is not None:
        print(f"HW exec time: {res.exec_time_ns} ns")
    return np.asarray(res.results[0]["out"]).astype(np.float32)
